# revision 1
# baseline (speedup 1.0000x reference)
"""MultiHeadAttention Trainium2 kernel (8-core batch-parallel), v4.

Reference computation (per batch b):
    K = k @ Wk + bk ; V = v @ Wv + bv ; Q = (q @ Wq + bq) * (1/8)
    per head h: scores = Qh @ Kh^T ; scores[mask!=0] = -inf
    attn = softmax(scores, axis=-1)
    context_h = attn @ Vh ; output = concat(context) @ Wo + bo
    attn_mean = sum_h(attn) / 16

Sharding: pure data-parallel over batch (B=8 -> one batch per core).

Per-core design ("transposed softmax"):
  - bf16 matmuls, fp32 PSUM. Projections produce Q^T/K^T ([d, s]; bias
    via ScalarE per-partition bias AP) and V_ext (natural [s, (h, 65)];
    col 64 = ones for fused row sums; V scaled by 16 so the 1/16
    head-mean factor is free).
  - Mask on PE: scoresT psum += I.T @ (-30000*maskT) -> exp gives 0.
    (MASK_DVE_KTS kts instead multiply exp by keepT on DVE.)
  - Head pairs: the two heads of a 128-partition block issue adjacent
    K=64 scores matmuls (PE row groups 0-1 / 2-3, concurrent).
  - All PSUM from two shared top-level pools (no phase barriers):
    psA = 4x 1-bank [128,512] slots, psB = 2x 2-bank [65,1024] slots.
  - exp on ScalarE per [128,512] bank; em kept bf16 for the mean path.
  - PV: ctxU^T[65, q] += Vx^T @ em per k tile; row 64 = 16*denoms.
  - Tail: sums -> [128,8] (PE) -> recip (DVE) -> transpose (PE) ->
    row-broadcast (PE) -> rb16 = 1/(16*sums) bf16.
  - ctx = ctxU*rb16; attn_mean accumulated transposed with plain bf16
    tensor_tensor muls/adds (DVE 2x mode); head 0 writes directly.
  - Epilogue: out = ctxT^T @ Wo + bo; attn_mean transposed on PE.
"""

import numpy as np

import concourse.bass as bass
import concourse.mybir as mybir
import concourse.tile as tile
from concourse import bacc
from concourse.masks import make_identity

F32 = mybir.dt.float32
BF16 = mybir.dt.bfloat16
I32 = mybir.dt.int32
AF = mybir.ActivationFunctionType
OP = mybir.AluOpType

B = 8
S = 1024
D = 1024
H = 16
DH = 64
P = 128

MASK_BIG = -30000.0  # representable in bf16; exp(s + MASK_BIG) == 0 in f32

# tuning knobs
MEAN_GPS_KTS = ()        # kt indices whose mean-accumulate runs on gpsimd
MASK_DVE_KTS = ()  # kts whose mask is DVE (mbT==0)*exp
DEBUG_DUMPS = False


def build_attention_nc(s=S, h=H, debug=False):
    d = D
    nt = d // P          # tiles along d (8)
    st = s // P          # tiles along s (8)
    hpt = P // DH        # heads per 128-partition tile (2)
    npair = h // hpt     # head pairs (8)

    nc = bacc.Bacc("TRN2", target_bir_lowering=False, debug=debug)

    dq = nc.dram_tensor("q", [s, d], F32, kind="ExternalInput")
    dk = nc.dram_tensor("k", [s, d], F32, kind="ExternalInput")
    dv = nc.dram_tensor("v", [s, d], F32, kind="ExternalInput")
    dmask = nc.dram_tensor("attn_mask", [s, s], I32, kind="ExternalInput")
    dWq = nc.dram_tensor("Wq", [d, d], F32, kind="ExternalInput")
    dWk = nc.dram_tensor("Wk", [d, d], F32, kind="ExternalInput")
    dWv = nc.dram_tensor("Wv", [d, d], F32, kind="ExternalInput")
    dWo = nc.dram_tensor("Wo", [d, d], F32, kind="ExternalInput")
    dbq = nc.dram_tensor("bq", [d], F32, kind="ExternalInput")
    dbk = nc.dram_tensor("bk", [d], F32, kind="ExternalInput")
    dbv = nc.dram_tensor("bv", [d], F32, kind="ExternalInput")
    dbo = nc.dram_tensor("bo", [d], F32, kind="ExternalInput")
    dout = nc.dram_tensor("output", [s, d], F32, kind="ExternalOutput")
    dmean = nc.dram_tensor("attn_mean", [s, s], F32, kind="ExternalOutput")
    ddbg = {}
    if DEBUG_DUMPS:
        for nm in ("QT", "KT", "ctxT"):
            ddbg[nm] = nc.dram_tensor(f"dbg_{nm}", [P, nt, s], BF16,
                                      kind="ExternalOutput")
        ddbg["Vx"] = nc.dram_tensor("dbg_Vx", [P, st, h, DH + 1], BF16,
                                    kind="ExternalOutput")
        ddbg["mbT"] = nc.dram_tensor("dbg_mbT", [P, st, s], BF16,
                                     kind="ExternalOutput")

    with tile.TileContext(nc) as tc:
        with (
            tc.tile_pool(name="persist", bufs=1) as persist,
            tc.tile_pool(name="consts", bufs=1) as consts,
            tc.tile_pool(name="dram", bufs=1, space="DRAM") as dram,
            tc.tile_pool(name="psA", bufs=4, space="PSUM") as psA,
            tc.tile_pool(name="psB", bufs=2, space="PSUM") as psB,
        ):
            # ---------- constants ----------
            identB = consts.tile([P, P], BF16)
            make_identity(nc, identB)
            ident_f = consts.tile([P, P], F32)
            make_identity(nc, ident_f)
            ones_row = consts.tile([1, s], BF16)
            nc.vector.memset(ones_row, 1.0)
            ones_f32 = consts.tile([1, 1], F32)
            nc.vector.memset(ones_f32, 1.0)
            # onehot[i, j, c] = (i == j), bf16: stationary for row-broadcasts
            onehot = consts.tile([st, st, P], BF16)
            nc.gpsimd.memset(onehot, 0.0)
            nc.gpsimd.affine_select(
                out=onehot, in_=onehot, compare_op=OP.not_equal, fill=1.0,
                base=0, pattern=[[-1, st], [0, P]], channel_multiplier=1,
            )

            # persistent big tensors
            QT = persist.tile([P, nt, s], BF16)
            KT = persist.tile([P, nt, s], BF16)
            Vx = persist.tile([P, st, h, DH + 1], BF16)
            mbT = persist.tile([P, st, s], BF16)   # (-30000 * mask)^T
            ctxT = persist.tile([P, nt, s], BF16)
            meanTs = []
            for kt in range(st):
                mtile = persist.tile([P, s], BF16, tag=f"meanT{kt}",
                                     name=f"meanT{kt}")
                meanTs.append(mtile)

            # per-partition bias columns for Q/K (ScalarE bias path)
            bq8 = consts.tile([P, nt], F32)
            bk_c = consts.tile([P, nt], F32)
            brows = {}

            # ---------- phase 0: load, cast, transpose, project ----------
            with (
                tc.tile_pool(name="stage", bufs=1) as stage,
                tc.tile_pool(name="xT", bufs=2) as xTp,
                tc.tile_pool(name="wpool", bufs=2) as wpool,
            ):
                # biases
                bqf = stage.tile([P, nt], F32, tag="bias_c", bufs=2)
                nc.sync.dma_start(out=bqf, in_=dbq.rearrange("(i p) -> p i", p=P))
                nc.vector.tensor_scalar(
                    out=bq8, in0=bqf, scalar1=1.0 / 8.0, scalar2=None,
                    op0=OP.mult,
                )
                bkf = stage.tile([P, nt], F32, tag="bias_c", bufs=2)
                nc.sync.dma_start(out=bkf, in_=dbk.rearrange("(i p) -> p i", p=P))
                nc.vector.tensor_copy(out=bk_c, in_=bkf)
                for nm, dt_ in (("bv", dbv), ("bo", dbo)):
                    rf = stage.tile([1, d], F32, tag="stage_w", bufs=2)
                    nc.sync.dma_start(out=rf, in_=dt_[None, :])
                    rb = consts.tile([1, d], BF16, tag=f"{nm}b")
                    nc.vector.tensor_copy(out=rb, in_=rf)
                    brows[nm] = rb

                def transposed_input(nm, src, queue, tg):
                    """DRAM f32 [s,d] -> SBUF bf16 [d,s] via cast + DRAM
                    bounce. All DMAs on one queue; DRAM->SBUF transpose
                    loads only after all stores on that queue."""
                    scratch = dram.tile([s, d], BF16, tag=f"sc_{nm}")
                    for c in range(st):
                        rows = slice(c * P, (c + 1) * P)
                        tf = stage.tile([P, d], F32,
                                        tag=f"f32_{tg}",
                                        bufs=3 if tg == "a" else 2,
                                        name=f"tf_{nm}{c}")
                        queue.dma_start(out=tf, in_=src[rows, :])
                        t16 = stage.tile([P, d], BF16,
                                         tag=f"bf_{tg}",
                                         bufs=3 if tg == "a" else 2,
                                         name=f"t16_{nm}{c}")
                        nc.vector.tensor_copy(out=t16, in_=tf)
                        queue.dma_start(out=scratch[rows, :], in_=t16)
                    xT = xTp.tile([P, nt, s], BF16, tag="xT")
                    for j in range(nt):
                        queue.dma_start_transpose(
                            out=xT[:, j, :], in_=scratch[:, j * P:(j + 1) * P]
                        )
                    return xT

                def load_weight_bf16(w_dram, queue, wtag="stage_w",
                                     wbufs=2):
                    wsb = wpool.tile([P, nt, d], BF16, tag="w")
                    for c in range(nt):
                        wf = stage.tile([P, d], F32, tag=wtag, bufs=wbufs,
                                        name=f"wf{c}")
                        queue.dma_start(out=wf, in_=w_dram[c * P:(c + 1) * P, :])
                        nc.scalar.copy(out=wsb[:, c, :], in_=wf)
                    return wsb

                def proj_T(wsb, x_T, outbuf, bias_col, scale):
                    """outbuf[dout, s] = ((x @ W) * scale + bias_col)."""
                    for mt in range(nt):
                        for cbi in range(2):
                            cb = slice(cbi * 512, (cbi + 1) * 512)
                            ps = psA.tile([P, 512], F32, tag="sc", name="pj")
                            for kt in range(nt):
                                nc.tensor.matmul(
                                    ps,
                                    lhsT=wsb[:, kt, mt * P:(mt + 1) * P],
                                    rhs=x_T[:, kt, cb],
                                    start=(kt == 0),
                                    stop=(kt == nt - 1),
                                )
                            nc.scalar.activation(
                                out=outbuf[:, mt, cb], in_=ps,
                                func=AF.Identity, scale=scale,
                                bias=bias_col[:, mt:mt + 1],
                            )

                qT_in = transposed_input("q", dq, nc.sync, "a")
                kT_in = transposed_input("k", dk, nc.sync, "b")
                wq = load_weight_bf16(dWq, nc.sync)
                wk = load_weight_bf16(dWk, nc.sync)
                proj_T(wq, qT_in, QT, bq8, 1.0 / 8.0)
                proj_T(wk, kT_in, KT, bk_c, 1.0)

                # mask: i32 -> -30000*mask bf16, transposed via bounce
                mscratch = dram.tile([s, s], BF16, tag="sc_mb")
                for c in range(st):
                    rows = slice(c * P, (c + 1) * P)
                    mi = stage.tile([P, s], I32,
                                    tag="f32_a", bufs=3, name=f"mi{c}")
                    nc.sync.dma_start(out=mi, in_=dmask[rows, :])
                    mb = stage.tile([P, s], BF16,
                                    tag="bf_a", bufs=3, name=f"mb{c}")
                    nc.vector.tensor_scalar(
                        out=mb, in0=mi, scalar1=1, scalar2=MASK_BIG,
                        op0=OP.is_equal, op1=OP.mult,
                    )
                    nc.sync.dma_start(out=mscratch[rows, :], in_=mb)
                for j in range(st):
                    nc.sync.dma_start_transpose(
                        out=mbT[:, j, :], in_=mscratch[:, j * P:(j + 1) * P]
                    )

                # V: natural layout, scaled by 16
                vT_in = transposed_input("v", dv, nc.sync, "a")
                wv = load_weight_bf16(dWv, nc.sync)
                for mt in range(st):
                    for cbi in range(2):
                        cb = slice(cbi * 512, (cbi + 1) * 512)
                        ps = psA.tile([P, 512], F32, tag="sc", name="pv_ps")
                        for kt in range(nt):
                            nc.tensor.matmul(
                                ps,
                                lhsT=vT_in[:, kt, mt * P:(mt + 1) * P],
                                rhs=wv[:, kt, cb],
                                start=(kt == 0),
                                stop=False,
                            )
                        nc.tensor.matmul(
                            ps,
                            lhsT=ones_row[0:1, 0:P],
                            rhs=brows["bv"][0:1, cb],
                            start=False,
                            stop=True,
                        )
                        nc.scalar.activation(
                            out=Vx[:, mt, cbi * 8:(cbi + 1) * 8, 0:DH],
                            in_=ps.rearrange("p (hh dh) -> p hh dh", dh=DH),
                            func=AF.Copy, scale=16.0,
                        )
                nc.vector.memset(Vx[:, :, :, DH:DH + 1], 1.0)
                if DEBUG_DUMPS:
                    for nm, t in (("QT", QT), ("KT", KT), ("mbT", mbT)):
                        nc.sync.dma_start(out=ddbg[nm][:, :, :], in_=t)
                    nc.sync.dma_start(out=ddbg["Vx"][:, :, :, :], in_=Vx)

            # ---------- head loop (pairs) ----------
            with (
                tc.tile_pool(name="emp", bufs=3) as emp,
                tc.tile_pool(name="attp", bufs=3) as attp,
                tc.tile_pool(name="rbp", bufs=3) as rbp,
                tc.tile_pool(name="hwork", bufs=2) as hwork,
            ):
                for pr in range(npair):
                    he, ho_ = 2 * pr, 2 * pr + 1
                    ems = {}
                    ppvs = {}
                    for hh in (he, ho_):
                        ems[hh] = emp.tile([P, st, s], BF16, tag="em",
                                           name=f"em{hh}")
                        ppvs[hh] = psB.tile([DH + 1, s], F32, tag="pv",
                                            name=f"pv{hh}")
                    for kt in range(st):
                        dve_mask = kt in MASK_DVE_KTS
                        for qbi in range(2):
                            cb = slice(qbi * 512, (qbi + 1) * 512)
                            pss = {}
                            # adjacent K=64 row-tile matmuls (concurrent)
                            for hh in (he, ho_):
                                lo = (hh % hpt) * DH
                                ps1 = psA.tile([P, 512], F32, tag="sc",
                                               name=f"sc{hh}")
                                nc.tensor.matmul(
                                    ps1,
                                    lhsT=KT[lo:lo + DH, pr,
                                            kt * P:(kt + 1) * P],
                                    rhs=QT[lo:lo + DH, pr, cb],
                                    start=True,
                                    stop=dve_mask,
                                )
                                pss[hh] = ps1
                            if not dve_mask:
                                for hh in (he, ho_):
                                    nc.tensor.matmul(
                                        pss[hh],
                                        lhsT=identB,
                                        rhs=mbT[:, kt, cb],
                                        start=False,
                                        stop=True,
                                    )
                            for hh in (he, ho_):
                                et = ems[hh][:, kt, cb]
                                nc.scalar.activation(
                                    out=et, in_=pss[hh], func=AF.Exp
                                )
                                if dve_mask:
                                    nc.vector.scalar_tensor_tensor(
                                        out=et, in0=mbT[:, kt, cb],
                                        scalar=0.0, in1=et,
                                        op0=OP.is_equal, op1=OP.mult,
                                    )
                                nc.tensor.matmul(
                                    ppvs[hh][:, cb],
                                    lhsT=Vx[:, kt, hh, :],
                                    rhs=et,
                                    start=(kt == 0),
                                    stop=(kt == st - 1),
                                )
                    # ---- per-head tail ----
                    for hh in (he, ho_):
                        lo = (hh % hpt) * DH
                        ppv = ppvs[hh]
                        srow = hwork.tile([1, s], F32, tag="srow")
                        nc.scalar.activation(
                            out=srow, in_=ppv[DH:DH + 1, :], func=AF.Copy,
                            scale=16.0,
                        )
                        p128 = psA.tile([P, st], F32, tag="sc", name="p128")
                        for j in range(st):
                            nc.tensor.matmul(
                                p128[:, j:j + 1],
                                lhsT=srow[0:1, j * P:(j + 1) * P],
                                rhs=ones_f32,
                                start=True, stop=True,
                            )
                        r128 = hwork.tile([P, st], F32, tag="r128")
                        nc.vector.reciprocal(out=r128, in_=p128)
                        rT_ps = psA.tile([st, P], F32, tag="sc", name="rT_ps")
                        nc.tensor.transpose(out=rT_ps, in_=r128,
                                            identity=ident_f)
                        rT = hwork.tile([st, P], BF16, tag="rT")
                        nc.scalar.activation(out=rT, in_=rT_ps, func=AF.Copy)
                        rb16 = rbp.tile([P, s], BF16, tag="rb")
                        for qbi in range(2):
                            prb = psA.tile([P, 512], F32, tag="sc",
                                           name="prb")
                            for j2 in range(4):
                                j = qbi * 4 + j2
                                nc.tensor.matmul(
                                    prb[:, j2 * P:(j2 + 1) * P],
                                    lhsT=onehot[:, j, :],
                                    rhs=rT,
                                    start=True, stop=True,
                                )
                            nc.scalar.activation(
                                out=rb16[:, qbi * 512:(qbi + 1) * 512],
                                in_=prb, func=AF.Copy,
                            )
                        nc.vector.tensor_tensor(
                            out=ctxT[lo:lo + DH, pr, :], in0=ppv[0:DH, :],
                            in1=rb16[0:DH, :], op=OP.mult,
                        )
                        em = ems[hh]
                        for kt in range(st):
                            eng = (nc.gpsimd if kt in MEAN_GPS_KTS
                                   else nc.vector)
                            if hh == 0:
                                eng.tensor_tensor(
                                    out=meanTs[kt], in0=em[:, kt, :],
                                    in1=rb16, op=OP.mult,
                                )
                            else:
                                at = attp.tile([P, s], BF16, tag="at")
                                eng.tensor_tensor(
                                    out=at, in0=em[:, kt, :], in1=rb16,
                                    op=OP.mult,
                                )
                                eng.tensor_tensor(
                                    out=meanTs[kt], in0=at, in1=meanTs[kt],
                                    op=OP.add,
                                )

            # ---------- epilogue ----------
            with (
                tc.tile_pool(name="osb", bufs=3) as osb,
                tc.tile_pool(name="wo", bufs=1) as wop,
            ):
                wo = wop.tile([P, nt, d], BF16)
                for c in range(nt):
                    wf = osb.tile([P, d], F32, tag="wof32", bufs=3)
                    nc.scalar.dma_start(out=wf, in_=dWo[c * P:(c + 1) * P, :])
                    nc.scalar.copy(out=wo[:, c, :], in_=wf)
                for mt in range(st):
                    oo = osb.tile([P, d], F32, tag="out_sb")
                    for cbi in range(2):
                        cb = slice(cbi * 512, (cbi + 1) * 512)
                        pso = psA.tile([P, 512], F32, tag="sc", name="pso")
                        for kt in range(nt):
                            nc.tensor.matmul(
                                pso,
                                lhsT=ctxT[:, kt, mt * P:(mt + 1) * P],
                                rhs=wo[:, kt, cb],
                                start=(kt == 0),
                                stop=False,
                            )
                        nc.tensor.matmul(
                            pso,
                            lhsT=ones_row[0:1, 0:P],
                            rhs=brows["bo"][0:1, cb],
                            start=False,
                            stop=True,
                        )
                        nc.scalar.activation(out=oo[:, cb], in_=pso,
                                             func=AF.Copy)
                    nc.sync.dma_start(out=dout[mt * P:(mt + 1) * P, :], in_=oo)

                    # attn_mean natural tile via PE transposes
                    mnat = osb.tile([P, s], F32, tag="mean_nat")
                    for kt in range(st):
                        pool_t = psB if kt % 2 == 0 else psA
                        tag_t = "pv" if kt % 2 == 0 else "sc"
                        tps = pool_t.tile([P, P], BF16, tag=tag_t, name="tps")
                        nc.tensor.transpose(
                            out=tps,
                            in_=meanTs[kt][:, mt * P:(mt + 1) * P],
                            identity=identB,
                        )
                        nc.vector.tensor_copy(
                            out=mnat[:, kt * P:(kt + 1) * P], in_=tps
                        )
                    nc.scalar.dma_start(
                        out=dmean[mt * P:(mt + 1) * P, :], in_=mnat
                    )

    nc.compile()
    return nc


_NC_CACHE = {}


def _get_nc():
    if "nc" not in _NC_CACHE:
        _NC_CACHE["nc"] = build_attention_nc()
    return _NC_CACHE["nc"]


def kernel(k, v, q, attn_mask, Wk, bk, Wv, bv, Wq, bq, Wo, bo, **_ignored):
    from concourse.bass_utils import run_bass_kernel_spmd

    k = np.asarray(k, np.float32)
    v = np.asarray(v, np.float32)
    q = np.asarray(q, np.float32)
    attn_mask = np.asarray(attn_mask, np.int32)
    shared = {
        "Wk": np.asarray(Wk, np.float32), "bk": np.asarray(bk, np.float32),
        "Wv": np.asarray(Wv, np.float32), "bv": np.asarray(bv, np.float32),
        "Wq": np.asarray(Wq, np.float32), "bq": np.asarray(bq, np.float32),
        "Wo": np.asarray(Wo, np.float32), "bo": np.asarray(bo, np.float32),
    }
    in_maps = []
    for b in range(B):
        m = {"q": q[b], "k": k[b], "v": v[b], "attn_mask": attn_mask[b]}
        m.update(shared)
        in_maps.append(m)

    nc = _get_nc()
    res = run_bass_kernel_spmd(nc, in_maps, core_ids=list(range(B)))
    output = np.stack([res.results[b]["output"] for b in range(B)])
    attn_mean = np.stack([res.results[b]["attn_mean"] for b in range(B)])
    return output, attn_mean

